# revision 36
# baseline (speedup 1.0000x reference)
"""Trainium2 Bass kernel for nn_EnhancedEdgeAwareGNN (edge-aware GAT, 6 layers).

Sharding: destination-node blocks (128 nodes) are assigned to 8 cores balanced
by in-edge count; each core aggregates all in-edges of its blocks (no
all-reduce), and one bf16 all-gather of h per layer republishes node features.
SPMD-uniform program: every core runs identical code; all per-core variation
lives in input tables (index tables, dstloc, permuted edge_attr).

Math reductions (exact): edge features enter only via al_e = w_edge @ Me
([128,24] per-layer reduction of lin_edge_w x att_edge); al_s/al_d likewise;
aggregation runs on h (128-d) with the per-head linear applied after; softmax
uses exp without max-subtraction (logits are O(1)); self-loops are dedicated
chunks fed by an all-reduced mean logit row.

Host<->device traffic is minimized (the axon tunnel at ~45 MB/s dominates the
measured wall): the initial node embedding is computed on own blocks only and
all-gathered (no replicated x); replicated [128,c] weights ship as bf16 16-row
slices (1/8 per core) and are re-assembled on-device with an AllGather over
the device fabric; broadcast-over-partition constants ship as single rows and
are expanded on-device via ones-row matmul; iota/identity are generated
on-device; the merged src|dst gather index table ships at 1/8 size (16
partitions) and is replicated to 128 partitions on-device; edge_attr ships as
int8 with the dequant scale folded into ea_proj_w; and the output returns as
per-row-scaled int8 (+f32 scales), dequantized on host.  The jax persistent
compilation cache plus a frozen BIR serialization keep the per-call
recompile/lowering overhead out of the repeat path.
"""

import sys

import numpy as np

sys.path.insert(0, "/opt/trn_rl_repo")

N, E, ND, ED, H, OUT, L, VC = 10000, 160000, 8, 4, 128, 256, 6, 6
HEADS, C = 4, 128
NCORES = 8
BLK = 128
NBLK_CORE = 10          # windows (128-node blocks) per core, padded
SPC = NBLK_CORE * BLK   # padded nodes per core (1280)
NPAD = NCORES * SPC     # padded rows in the replicated h table (10240)
SC_CHUNKS = 38          # chunks per gather superchunk (190 = 5x38, zero pad)

# offsets into the packed broadcast-rows input [1, ROWS_TOT]
R_GB, R_G, R_B = 0, L * H, 2 * L * H
R_NB, R_OUTB = 3 * L * H, 3 * L * H + H
ROWS_TOT = 3 * L * H + H + OUT

# replicated [128, cols] weights ship as 16-row slices (1/8 per core) and are
# re-assembled on-device with an AllGather over the fast device fabric
PACK = [("lw_all", L * HEADS * C), ("att1_top", H), ("att2", 64), ("outw", OUT),
        ("msd", L * 8), ("me", L * HEADS), ("b1p", 1), ("eab", 1)]
PCOLS = sum(c for _, c in PACK)
POFF = {}
_o = 0
for _n, _c in PACK:
    POFF[_n] = _o
    _o += _c


# ----------------------------------------------------------------- host prep
def _split_blocks(dst):
    nblk = (N + BLK - 1) // BLK  # 79
    cnt = np.bincount(dst // BLK, minlength=nblk).astype(np.int64)
    cum = np.concatenate([[0], np.cumsum(cnt)])
    bounds = [0]
    for c in range(1, NCORES):
        target = cum[-1] * c / NCORES
        b = int(np.searchsorted(cum, target))
        lo = bounds[-1] + 1
        lo = max(lo, nblk - (NCORES - c) * NBLK_CORE)   # leave room behind
        hi = min(bounds[-1] + NBLK_CORE, nblk - (NCORES - c))
        bounds.append(max(lo, min(b, hi)))
    bounds.append(nblk)
    assert all(0 < bounds[i + 1] - bounds[i] <= NBLK_CORE for i in range(NCORES))
    return bounds


def _pad_coord(n, bounds):
    n = np.asarray(n)
    g = n // BLK
    c = np.searchsorted(np.asarray(bounds[1:]), g, side="right")
    return c * SPC + (g - np.asarray(bounds)[c]) * BLK + (n % BLK)


def _wrap16(idx):
    x = len(idx) // 16
    return np.ascontiguousarray(idx.reshape(x, 16).T.astype(np.int16))


def _build_graph(edge_index):
    src = np.asarray(edge_index[0], dtype=np.int64)
    dst = np.asarray(edge_index[1], dtype=np.int64)
    bounds = _split_blocks(dst)

    order = np.argsort(dst, kind="stable")
    src_s, dst_s = src[order], dst[order]
    blk_of = dst_s // BLK
    blk_starts = np.searchsorted(blk_of, np.arange(80))
    blk_ends = np.searchsorted(blk_of, np.arange(80), side="right")

    nblk = (N + BLK - 1) // BLK
    treg = max((blk_ends[g] - blk_starts[g] + BLK - 1) // BLK for g in range(nblk))
    T = treg + 1
    cht = NBLK_CORE * T
    cht_pad = ((cht + SC_CHUNKS - 1) // SC_CHUNKS) * SC_CHUNKS
    nsc = cht_pad // SC_CHUNKS

    pc_src = _pad_coord(src_s, bounds)

    cores = []
    for c in range(NCORES):
        src_idx = np.zeros(cht_pad * BLK, dtype=np.int64)
        dst_idx = np.zeros(cht_pad * BLK, dtype=np.int64)
        dstloc = np.full((BLK, cht_pad), -1.0, dtype=np.float32)
        nregs = NBLK_CORE * treg
        ea_perm = np.zeros((nregs * BLK,), dtype=np.int64)
        ea_mask = np.zeros((nregs * BLK,), dtype=bool)
        for j in range(NBLK_CORE):
            g = bounds[c] + j
            real = g < bounds[c + 1]
            wbase = c * SPC + j * BLK
            cnt = (blk_ends[g] - blk_starts[g]) if real else 0
            s0 = blk_starts[g] if real else 0
            for k in range(treg):
                ch = j * T + k
                e0 = k * BLK
                take = max(0, min(BLK, cnt - e0))
                pos = ch * BLK
                if take:
                    sl = slice(s0 + e0, s0 + e0 + take)
                    src_idx[pos:pos + take] = pc_src[sl]
                    dst_idx[pos:pos + take] = wbase + (dst_s[sl] - g * BLK)
                    dstloc[:take, ch] = (dst_s[sl] - g * BLK).astype(np.float32)
                    gp = (j * treg + k) * BLK
                    ea_perm[gp:gp + take] = order[sl]
                    ea_mask[gp:gp + take] = True
            # self-loop chunk: real nodes gather themselves, pads gather row 0
            ch = j * T + treg
            pos = ch * BLK
            ids = np.arange(BLK)
            nreal = min(BLK, max(0, N - g * BLK)) if real else 0
            coords = np.where(ids < nreal, wbase + ids, 0)
            src_idx[pos:pos + BLK] = coords
            dst_idx[pos:pos + BLK] = coords
            dstloc[:nreal, ch] = ids[:nreal].astype(np.float32)
        import ml_dtypes
        scb = SC_CHUNKS * BLK
        merged = np.concatenate([
            np.concatenate([src_idx[s * scb:(s + 1) * scb],
                            dst_idx[s * scb:(s + 1) * scb]])
            for s in range(cht_pad // SC_CHUNKS)])
        cores.append(dict(sd_tab=_wrap16(merged),
                          dstloc=dstloc.astype(ml_dtypes.bfloat16),
                          ea_perm=ea_perm, ea_mask=ea_mask))
    meta = dict(bounds=bounds, T=int(T), treg=int(treg), cht_pad=int(cht_pad),
                nsc=int(nsc), nregs=int(NBLK_CORE * treg))
    return cores, meta


def _derive_weights(inp, meta, cores):
    f32 = np.float32
    gw = {}
    lw = np.asarray(inp["gat_lin_w"], f32).reshape(L, H, HEADS, C)
    lew = np.asarray(inp["gat_lin_edge_w"], f32).reshape(L, H, HEADS, C)
    Ms = np.einsum("lkhc,lhc->lkh", lw, np.asarray(inp["gat_att_src"], f32))
    Md = np.einsum("lkhc,lhc->lkh", lw, np.asarray(inp["gat_att_dst"], f32))
    Me = np.einsum("lkhc,lhc->lkh", lew, np.asarray(inp["gat_att_edge"], f32))
    packed = {}
    packed["msd"] = np.ascontiguousarray(
        np.concatenate([Ms, Md], axis=2).transpose(1, 0, 2)).astype(f32)   # [128,L,8]
    packed["me"] = np.ascontiguousarray(Me.transpose(1, 0, 2).reshape(H, L * HEADS)).astype(f32)
    packed["lw_all"] = np.ascontiguousarray(
        (np.asarray(inp["gat_lin_w"], f32) * 0.25).transpose(1, 0, 2)).astype(f32)  # [128,L,512]
    rows = np.zeros((1, ROWS_TOT), f32)
    rows[0, R_GB:R_GB + L * H] = np.asarray(inp["gat_bias"], f32).ravel()
    rows[0, R_G:R_G + L * H] = np.asarray(inp["ln_scale"], f32).ravel()
    rows[0, R_B:R_B + L * H] = np.asarray(inp["ln_bias"], f32).ravel()
    rows[0, R_NB:R_NB + H] = np.asarray(inp["node_b"], f32).ravel()
    rows[0, R_OUTB:R_OUTB + OUT] = np.asarray(inp["out_b"], f32).ravel()
    gw["rows"] = rows
    vnf = np.asarray(inp["vnf_context"], f32) @ np.asarray(inp["vnf_w"], f32) \
        + np.asarray(inp["vnf_b"], f32)
    att1 = np.asarray(inp["att1_w"], f32)
    packed["b1p"] = (np.asarray(inp["att1_b"], f32) + (vnf @ att1[H:]).ravel()).reshape(H, 1)
    packed["att1_top"] = np.ascontiguousarray(att1[:H])
    packed["att2"] = np.asarray(inp["att2_w"], f32)
    gw["b2a3"] = np.concatenate([np.asarray(inp["att2_b"], f32).reshape(-1, 1),
                                 np.asarray(inp["att3_w"], f32)], axis=1)
    meta["b3"] = float(np.asarray(inp["att3_b"], f32).ravel()[0])
    import ml_dtypes
    gw["ea_proj_w"] = (np.asarray(inp["ea_proj_w"], f32) / 127.0).astype(ml_dtypes.bfloat16)
    packed["eab"] = np.asarray(inp["ea_proj_b"], f32).reshape(H, 1)

    packed["outw"] = np.asarray(inp["out_w"], f32)

    wfull = np.zeros((H, PCOLS), f32)
    for name, cols in PACK:
        wfull[:, POFF[name]:POFF[name] + cols] = packed[name].reshape(H, cols)
    meta["wfull"] = wfull.astype(ml_dtypes.bfloat16)

    bounds = meta["bounds"]
    x = np.asarray(inp["x"], f32)
    xT = np.zeros((ND, NPAD), f32)
    xT[:, _pad_coord(np.arange(N), bounds)] = x.T
    # per-core own x (local padded coords) for the fp32 h kept on-chip
    nwq = np.asarray(inp["node_w"], f32)
    for c, cd in enumerate(cores):
        cd["xT_own"] = np.ascontiguousarray(
            np.concatenate([xT[:, c * SPC:(c + 1) * SPC], nwq], axis=1)
        ).astype(ml_dtypes.bfloat16)
        buf = np.zeros((ED, meta["nregs"] * BLK), f32)
        m = cd["ea_mask"]
        buf[:, m] = np.asarray(inp["edge_attr"], f32)[cd["ea_perm"][m]].T
        cd["eaT"] = np.rint(buf * 127.0).clip(-127, 127).astype(np.int8)
    return gw


# --------------------------------------------------------------- bass kernel
def _build_kernel(meta):
    import os as _os
    STAGE = int(_os.environ.get("K_STAGE", "3"))
    SKIP = _os.environ.get("K_SKIP", "none")
    NL = int(_os.environ.get("K_NLAYERS", str(L)))
    import concourse.bass as bass
    import concourse.bacc as bacc
    import concourse.tile as tile
    from concourse import mybir

    F32, BF16, I16, I32 = (mybir.dt.float32, mybir.dt.bfloat16, mybir.dt.int16,
                           mybir.dt.int32)
    AF = mybir.ActivationFunctionType
    ALU = mybir.AluOpType
    T, treg = meta["T"], meta["treg"]
    cht_pad, nsc, nregs = meta["cht_pad"], meta["nsc"], meta["nregs"]
    B3 = meta["b3"]
    RG = [list(range(NCORES))]

    nc = bacc.Bacc(num_devices=NCORES)
    nc.has_collectives = True

    def ein(name, shape, dt=F32):
        return nc.dram_tensor(name, shape, dt, kind="ExternalInput")

    I8 = mybir.dt.int8
    F32TOT = ROWS_TOT + 64 * 2
    O_DL, N_DL = 0, BLK * cht_pad
    O_XT, N_XT = N_DL, ND * (SPC + H)
    O_WP, N_WP = O_XT + N_XT, 16 * PCOLS
    O_EW, N_EW = O_WP + N_WP, ED * H
    BFTOT = O_EW + N_EW
    SDTOT = 16 * 2 * cht_pad * 8
    EATOT = ED * nregs * BLK
    B_BF = F32TOT * 4                 # byte offsets of each segment
    B_SD = B_BF + BFTOT * 2
    B_EA = B_SD + SDTOT * 2
    BTOT = B_EA + EATOT
    ball_d = ein("ball", [1, BTOT], I8)
    blob_f32_d = ball_d[0:1, 0:F32TOT * 4].bitcast(F32)
    blob_bf_d = ball_d[0:1, B_BF:B_BF + BFTOT * 2].bitcast(BF16)
    sd_tab_src = (ball_d[0:1, B_SD:B_SD + SDTOT * 2].bitcast(I16)
                  .rearrange("o (p c) -> (o p) c", p=16))
    eaT_d = (ball_d[0:1, B_EA:B_EA + EATOT].rearrange("o (p c) -> (o p) c", p=ED))

    out_d = nc.dram_tensor("out", [SPC, OUT + 4], I8, kind="ExternalOutput")

    h_table = nc.dram_tensor("h_table", [NPAD, H], BF16, addr_space="Shared")
    own_slice = nc.dram_tensor("own_slice", [SPC, H], BF16)
    alel_in = nc.dram_tensor("alel_in", [1, L * HEADS], F32)
    alel_out = nc.dram_tensor("alel_out", [1, L * HEADS], F32, addr_space="Shared")
    wpart_i = nc.dram_tensor("wpart_i", [16, PCOLS], BF16)
    wfull_d = nc.dram_tensor("wfull", [NCORES, 16, PCOLS], BF16, addr_space="Shared")

    with tile.TileContext(nc) as tc:
        with (
            tc.tile_pool(name="consts", bufs=1) as cp,
            tc.tile_pool(name="persist", bufs=1) as pers,
            tc.tile_pool(name="gath", bufs=2) as gp,
            tc.tile_pool(name="work", bufs=3) as wp,
            tc.tile_pool(name="small", bufs=4) as sp,
            tc.tile_pool(name="ps2", bufs=2, space="PSUM") as ps2,
            tc.tile_pool(name="ps1", bufs=1, space="PSUM") as ps1,
        ):
            def cload(dram, dt=None, name=None):
                t = cp.tile(dram.shape, dt or dram.dtype, name=name or (dram.name + "_sb"))
                nc.sync.dma_start(t[:], dram[:])
                return t

            def bfload(off, shape, name):
                t = cp.tile(shape, BF16, name=name)
                nc.sync.dma_start(
                    t[:], blob_bf_d[:, off:off + shape[0] * shape[1]]
                    .rearrange("o (p c) -> (o p) c", p=shape[0]))
                return t

            dstloc = bfload(O_DL, [BLK, cht_pad], "dstloc_sb")
            xTon = bfload(O_XT, [ND, SPC + H], "xTon_sb")
            eaw = bfload(O_EW, [ED, H], "eaw_sb")
            rows_sb = cp.tile([1, ROWS_TOT], F32, name="rows_sb")
            nc.sync.dma_start(rows_sb[:], blob_f32_d[:, 0:ROWS_TOT])
            b2a3 = cp.tile([64, 2], F32, name="b2a3_sb")
            nc.sync.dma_start(
                b2a3[:], blob_f32_d[:, ROWS_TOT:ROWS_TOT + 128]
                .rearrange("o (p c) -> (o p) c", p=64))

            # re-assemble the replicated weights from the 8 per-core slices
            # (collectives cannot read IO tensors; stage through internal DRAM)
            nc.sync.dma_start(
                wpart_i[:], blob_bf_d[:, O_WP:O_WP + N_WP]
                .rearrange("o (p c) -> (o p) c", p=16))
            nc.gpsimd.collective_compute(
                "AllGather", ALU.bypass, replica_groups=RG,
                ins=[wpart_i[:]], outs=[wfull_d[:]])

            def pload(name, shape):
                t = cp.tile(shape, BF16, name=name + "_sb")
                flat = t[:] if len(shape) == 2 else t[:].rearrange("p a b -> p (a b)")
                off = POFF[name]
                cols = dict(PACK)[name]
                for k in range(8):
                    nc.sync.dma_start(flat[16 * k:16 * (k + 1), :],
                                      wfull_d[k, :, off:off + cols])
                return t

            lw = pload("lw_all", [H, L, HEADS * C])
            att1 = pload("att1_top", [H, H])
            att2 = pload("att2", [H, 64])
            outw = pload("outw", [H, OUT])
            msd = pload("msd", [H, L, 8])
            me = pload("me", [H, L * HEADS])
            b1p_h = pload("b1p", [H, 1])
            eab_h = pload("eab", [H, 1])
            b1p = cp.tile([H, 1], F32, name="b1p_f")
            nc.vector.tensor_copy(b1p[:], b1p_h[:])
            eab = cp.tile([H, 1], F32, name="eab_f")
            nc.vector.tensor_copy(eab[:], eab_h[:])
            att3b = cp.tile([64, 1], BF16, name="att3_bf")
            nc.vector.tensor_copy(att3b[:], b2a3[:, 1:2])

            # index table: ship 16 partitions, replicate 8x on-device
            sd_tab = cp.tile([BLK, 2 * cht_pad * 8], I16, name="sd_tab_sb")
            for k in range(8):
                nc.sync.dma_start(sd_tab[16 * k:16 * (k + 1), :], sd_tab_src)

            for cv in (0.0, 1e-5, B3):
                ct = cp.tile([BLK, 1], F32, name=f"const_{abs(hash(cv)) % 10**8}")
                nc.vector.memset(ct[:], cv)
                nc.const_aps.aps[(F32, cv)] = ct[:]

            ones_bf = cp.tile([BLK, 1], BF16, name="ones_bf")
            nc.vector.memset(ones_bf[:], 1.0)
            one_f = cp.tile([1, 1], F32, name="one_f")
            nc.vector.memset(one_f[:], 1.0)
            ones_row = cp.tile([1, BLK], F32, name="ones_row")
            nc.vector.memset(ones_row[:], 1.0)

            # on-device iota (0..127 along free dim, same per partition)
            iota_i = cp.tile([BLK, BLK], I32, name="iota_i")
            nc.gpsimd.iota(iota_i[:], pattern=[[1, BLK]], base=0, channel_multiplier=0)
            iota = cp.tile([BLK, BLK], BF16, name="iota")
            nc.vector.tensor_copy(iota[:], iota_i[:])
            # on-device identity: keep ones where (col - partition) == 0
            ident = cp.tile([BLK, BLK], F32, name="ident")
            nc.vector.memset(ident[:], 1.0)
            nc.gpsimd.affine_select(ident[:], ident[:], pattern=[[1, BLK]],
                                    compare_op=ALU.is_equal, fill=0.0, base=0,
                                    channel_multiplier=-1)
            ident_bf = cp.tile([BLK, BLK], BF16, name="ident_bf")
            nc.vector.tensor_copy(ident_bf[:], ident[:])

            # broadcast packed rows across 128 partitions via ones-row matmul
            gbias = cp.tile([BLK, L, H], F32, name="gbias")
            grep = cp.tile([BLK, L, H], F32, name="grep")
            brep = cp.tile([BLK, L, H], F32, name="brep")
            nbrep = cp.tile([BLK, H], F32, name="nbrep")
            outbrep = cp.tile([BLK, OUT], F32, name="outbrep")

            def bcast(dst_ap, off, n):
                for c0 in range(0, n, 384):
                    cn = min(384, n - c0)
                    bp = ps2.tile([BLK, 384], F32, name="bcast_ps", tag="big", bufs=3)
                    nc.tensor.matmul(bp[:, :cn], ones_row[:],
                                     rows_sb[:, off + c0:off + c0 + cn],
                                     start=True, stop=True)
                    nc.vector.tensor_copy(dst_ap[:, c0:c0 + cn], bp[:, :cn])

            bcast(gbias[:].rearrange("p a b -> p (a b)"), R_GB, L * H)
            bcast(grep[:].rearrange("p a b -> p (a b)"), R_G, L * H)
            bcast(brep[:].rearrange("p a b -> p (a b)"), R_B, L * H)
            bcast(nbrep[:], R_NB, H)
            bcast(outbrep[:], R_OUTB, OUT)

            h_own = pers.tile([BLK, NBLK_CORE, H], F32, name="h_own")
            al_e = pers.tile([BLK, nregs, L * HEADS], F32, name="al_e")
            alel_sb = pers.tile([BLK, L * HEADS], F32, name="alel_sb")

            # ---- P0: initial embedding on own blocks; all-gather the table
            for w in range(NBLK_CORE):
                h0p = ps2.tile([BLK, 4, H], F32, name="hps", tag="big", bufs=3)
                nc.tensor.matmul(h0p[:, 0, :], xTon[:, w * BLK:(w + 1) * BLK],
                                 xTon[:, SPC:SPC + H], start=True, stop=True)
                nc.vector.tensor_tensor(out=h_own[:, w, :], in0=h0p[:, 0, :], in1=nbrep[:],
                                        op=ALU.add)
                h0b = wp.tile([BLK, H], BF16, name="h0b", tag="h0b")
                nc.vector.tensor_copy(h0b[:], h_own[:, w, :])
                nc.sync.dma_start(own_slice[w * BLK:(w + 1) * BLK, :], h0b[:])
            nc.gpsimd.collective_compute(
                "AllGather", ALU.bypass, replica_groups=RG,
                ins=[own_slice[:]], outs=[h_table[:]])

            # ---- P1: edge gate MLP -> al_e table; masked column-sum -> allreduce
            GG = 4
            if STAGE < 1:
                nc.vector.memset(al_e[:], 0.0)
                nc.vector.memset(alel_sb[:], 0.0)
            if STAGE >= 1:
                alel_ps = ps1.tile([1, L * HEADS], F32, name="alel_ps", tag="alel")
                for rc0 in range(0, nregs, GG):
                    gn = min(GG, nregs - rc0)
                    gw_ = gn * BLK
                    ea_q = wp.tile([ED, GG * BLK], mybir.dt.int8, name="ea_q", tag="ea_q")
                    nc.sync.dma_start(ea_q[:, :gw_], eaT_d[:, rc0 * BLK:(rc0 + gn) * BLK])
                    ea_t = wp.tile([ED, GG * BLK], BF16, name="ea_t", tag="ea_t")
                    nc.vector.tensor_copy(ea_t[:, :gw_], ea_q[:, :gw_])
                    efp = ps2.tile([BLK, GG * BLK], F32, name="efp", tag="big", bufs=3)
                    nc.tensor.matmul(efp[:, :gw_], eaw[:], ea_t[:, :gw_], start=True, stop=True)
                    efb = wp.tile([BLK, GG * BLK], BF16, name="efb", tag="efb")
                    nc.scalar.activation(efb[:, :gw_], efp[:, :gw_], AF.Identity, bias=eab[:, 0:1])
                    a1p = ps2.tile([BLK, GG * BLK], F32, name="a1p", tag="big", bufs=3)
                    nc.tensor.matmul(a1p[:, :gw_], att1[:], efb[:, :gw_], start=True, stop=True)
                    a1 = wp.tile([BLK, GG * BLK], BF16, name="a1", tag="a1")
                    nc.scalar.activation(a1[:, :gw_], a1p[:, :gw_], AF.Relu, bias=b1p[:, 0:1])
                    a2p = ps2.tile([64, GG * BLK], F32, name="a2p", tag="big", bufs=3)
                    nc.tensor.matmul(a2p[:, :gw_], att2[:], a1[:, :gw_], start=True, stop=True)
                    a2 = wp.tile([64, GG * BLK], BF16, name="a2", tag="a2")
                    nc.scalar.activation(a2[:, :gw_], a2p[:, :gw_], AF.Relu, bias=b2a3[:, 0:1])
                    for q in range(gn):
                        rc = rc0 + q
                        w = rc // treg
                        cg = w * T + (rc % treg)
                        gcp = ps2.tile([BLK, 32], F32, name="gcp", tag="big", bufs=3)
                        nc.tensor.matmul(gcp[:, 0:1], a2[:, q * BLK:(q + 1) * BLK],
                                         att3b[:], start=True, stop=True)
                        gcol = sp.tile([BLK, 1], F32, name="gcol", tag="gcol")
                        nc.scalar.activation(gcol[:], gcp[:, 0:1], AF.Sigmoid, bias=B3)
                        pfxp = ps2.tile([BLK, 32], F32, name="pfxp", tag="big", bufs=3)
                        nc.tensor.matmul(pfxp[:, 0:L * HEADS], efb[:, q * BLK:(q + 1) * BLK],
                                         me[:], start=True, stop=True)
                        nc.vector.tensor_scalar(out=al_e[:, rc, :], in0=pfxp[:, 0:L * HEADS],
                                                scalar1=gcol[:, 0:1], scalar2=None, op0=ALU.mult)
                        mask = sp.tile([BLK, 1], F32, name="mask", tag="mask")
                        nc.vector.tensor_scalar(out=mask[:], in0=dstloc[:, cg:cg + 1],
                                                scalar1=0.0, scalar2=None, op0=ALU.is_ge)
                        nc.tensor.matmul(alel_ps[:], mask[:], al_e[:, rc, :],
                                         start=(rc == 0), stop=(rc == nregs - 1))
                alel_row = sp.tile([1, L * HEADS], F32, name="alel_row")
                nc.vector.tensor_copy(alel_row[:], alel_ps[:])
                nc.sync.dma_start(alel_in[:], alel_row[:])
                nc.gpsimd.collective_compute(
                    "AllReduce", ALU.add, replica_groups=RG,
                    ins=[alel_in[:]], outs=[alel_out[:]])
                alel_row2 = sp.tile([1, L * HEADS], F32, name="alel_row2")
                nc.sync.dma_start(alel_row2[:], alel_out[:])
                alel_bp = ps1.tile([BLK, L * HEADS], F32, name="alel_bp", tag="alel")
                nc.tensor.matmul(alel_bp[:], ones_row[:], alel_row2[:], start=True, stop=True)
                nc.vector.tensor_scalar(out=alel_sb[:], in0=alel_bp[:], scalar1=1.0 / E,
                                        scalar2=None, op0=ALU.mult)

            # ---- P2: GAT layers
            ni_reg = nc.gpsimd.alloc_register()
            nc.gpsimd.reg_mov(ni_reg, SC_CHUNKS * BLK)
            ni2_reg = nc.gpsimd.alloc_register()
            nc.gpsimd.reg_mov(ni2_reg, 2 * SC_CHUNKS * BLK)
            for li in range(NL if STAGE >= 2 else 0):
                l = li % L

                gtiles = {}

                SCW = SC_CHUNKS * 8

                def issue_sc(sc):
                    if sc in gtiles:
                        return gtiles[sc]
                    hg = gp.tile([BLK, SC_CHUNKS, H], BF16, name=f"hg_{l}_{sc}",
                                 tag="hg", bufs=3)
                    hx = gp.tile([BLK, 1, 2 * SC_CHUNKS * BLK], BF16,
                                 name=f"hx_{l}_{sc}", tag="hx", bufs=3)
                    ssl = sd_tab[:, sc * 2 * SCW:sc * 2 * SCW + SCW]
                    msl = sd_tab[:, sc * 2 * SCW:(sc + 1) * 2 * SCW]
                    if SKIP == "gath":
                        gtiles[sc] = (hg, hx)
                        return gtiles[sc]
                    nc.gpsimd.dma_gather(out_ap=hx[:, :, :], in_ap=h_table[:, :], idxs_ap=msl,
                                         num_idxs=2 * SC_CHUNKS * BLK, num_idxs_reg=ni2_reg,
                                         elem_size=H, transpose=True, single_packet=False)
                    nc.gpsimd.dma_gather(out_ap=hg[:, :, :], in_ap=h_table[:, :], idxs_ap=ssl,
                                         num_idxs=SC_CHUNKS * BLK, num_idxs_reg=ni_reg,
                                         elem_size=H, single_packet=False)
                    gtiles[sc] = (hg, hx)
                    return gtiles[sc]

                def get_sc(sc):
                    t = issue_sc(sc)
                    if sc + 1 < nsc:
                        issue_sc(sc + 1)
                    return t

                for w in range(NBLK_CORE):
                    if SKIP == "body":
                        for k in range(T):
                            get_sc((w * T + k) // SC_CHUNKS)
                        continue
                    alpha_ps = ps2.tile([BLK, 4 * T], F32, name="alpha_ps", tag="big", bufs=3)
                    for k in range(T):
                        cg = w * T + k
                        sc, off = cg // SC_CHUNKS, cg % SC_CHUNKS
                        _, hx = get_sc(sc)
                        nc.tensor.matmul(alpha_ps[:, k * 4:(k + 1) * 4],
                                         hx[:, 0, off * BLK:(off + 1) * BLK],
                                         msd[:, l, 0:4], start=True, stop=False)
                        nc.tensor.matmul(alpha_ps[:, k * 4:(k + 1) * 4],
                                         hx[:, 0, (SC_CHUNKS + off) * BLK:(SC_CHUNKS + off + 1) * BLK],
                                         msd[:, l, 4:8], start=False, stop=True)
                    t_sb = wp.tile([BLK, 4 * T], F32, name="t_sb", tag="t_sb")
                    nc.vector.tensor_tensor(
                        out=t_sb[:, 0:4 * treg].rearrange("p (t f) -> p t f", f=4),
                        in0=alpha_ps[:, 0:4 * treg].rearrange("p (t f) -> p t f", f=4),
                        in1=al_e[:, w * treg:(w + 1) * treg, l * 4:(l + 1) * 4],
                        op=ALU.add)
                    nc.vector.tensor_tensor(out=t_sb[:, 4 * treg:4 * T],
                                            in0=alpha_ps[:, 4 * treg:4 * T],
                                            in1=alel_sb[:, l * 4:(l + 1) * 4], op=ALU.add)
                    u_sb = wp.tile([BLK, 4 * T], F32, name="u_sb", tag="u_sb")
                    nc.scalar.activation(u_sb[:], t_sb[:], AF.Lrelu, alpha=0.2)
                    ex_sb = wp.tile([BLK, 4 * T], BF16, name="ex_sb", tag="ex_sb")
                    nc.scalar.activation(ex_sb[:], u_sb[:], AF.Exp)

                    numT_ps = ps2.tile([BLK, HEADS * BLK], F32, name="numT_ps", tag="numT", bufs=2)
                    den_ps = ps2.tile([1, HEADS * BLK], F32, name="den_ps", tag="den", bufs=1)
                    for k in range(T):
                        cg = w * T + k
                        sc, off = cg // SC_CHUNKS, cg % SC_CHUNKS
                        hg, _ = get_sc(sc)
                        eq = wp.tile([BLK, BLK], BF16, name="eq", tag="eq")
                        nc.vector.tensor_tensor(out=eq[:],
                                                in0=dstloc[:, cg:cg + 1].to_broadcast([BLK, BLK]),
                                                in1=iota[:], op=ALU.is_equal)
                        sw = wp.tile([BLK, HEADS, BLK], BF16, name="sw", tag="sw")
                        nc.vector.tensor_tensor(
                            out=sw[:],
                            in0=eq[:].rearrange("p (o n) -> p o n", o=1)
                                     .to_broadcast([BLK, HEADS, BLK]),
                            in1=ex_sb[:, k * 4:(k + 1) * 4]
                                     .rearrange("p (h o) -> p h o", o=1)
                                     .to_broadcast([BLK, HEADS, BLK]),
                            op=ALU.mult)
                        nc.tensor.matmul(numT_ps[:], hg[:, off, :], sw[:, :, :],
                                         start=(k == 0), stop=(k == T - 1))
                        nc.tensor.matmul(den_ps[:], ones_bf[:], sw[:, :, :],
                                         start=(k == 0), stop=(k == T - 1))

                    numT_sb = wp.tile([BLK, HEADS * BLK], BF16, name="numT_sb", tag="numsb")
                    nc.vector.tensor_copy(numT_sb[:], numT_ps[:])
                    den_sb = sp.tile([1, HEADS * BLK], F32, name="den_sb", tag="densb")
                    nc.vector.tensor_copy(den_sb[:], den_ps[:])
                    denT_ps = ps2.tile([BLK, 4], F32, name="denT_ps", tag="big", bufs=3)
                    for hd in range(HEADS):
                        nc.tensor.matmul(denT_ps[:, hd:hd + 1],
                                         den_sb[:, hd * BLK:(hd + 1) * BLK], one_f[:],
                                         start=True, stop=True)
                    dr = sp.tile([BLK, 4], F32, name="dr", tag="dr")
                    nc.vector.tensor_scalar(out=dr[:], in0=denT_ps[:], scalar1=1e-30,
                                            scalar2=None, op0=ALU.add)
                    nc.vector.reciprocal(dr[:], dr[:])

                    hc_ps = ps2.tile([BLK, HEADS, BLK], F32, name="hc_ps", tag="big", bufs=3)
                    for hd in range(HEADS):
                        nc.tensor.matmul(hc_ps[:, hd, :],
                                         numT_sb[:, hd * BLK:(hd + 1) * BLK],
                                         lw[:, l, hd * C:(hd + 1) * C], start=True, stop=True)
                    acc = wp.tile([BLK, BLK], F32, name="acc", tag="acc")
                    nc.vector.tensor_scalar(out=acc[:], in0=hc_ps[:, 0, :],
                                            scalar1=dr[:, 0:1], scalar2=None, op0=ALU.mult)
                    for hd in range(1, HEADS):
                        tmp = sp.tile([BLK, BLK], F32, name="tmp", tag="tmp")
                        nc.vector.tensor_scalar(out=tmp[:], in0=hc_ps[:, hd, :],
                                                scalar1=dr[:, hd:hd + 1], scalar2=None,
                                                op0=ALU.mult)
                        nc.vector.tensor_tensor(out=acc[:], in0=acc[:], in1=tmp[:], op=ALU.add)
                    nc.vector.tensor_tensor(out=acc[:], in0=acc[:], in1=gbias[:, l, :], op=ALU.add)
                    nc.scalar.activation(acc[:], acc[:], AF.Relu)
                    r = wp.tile([BLK, BLK], F32, name="r", tag="r")
                    nc.vector.tensor_tensor(out=r[:], in0=acc[:], in1=h_own[:, w, :], op=ALU.add)
                    # LayerNorm over features
                    s1 = sp.tile([BLK, 1], F32, name="s1", tag="s1")
                    nc.vector.tensor_reduce(s1[:], r[:], axis=mybir.AxisListType.X, op=ALU.add)
                    negm = sp.tile([BLK, 1], F32, name="negm", tag="negm")
                    nc.scalar.activation(negm[:], s1[:], AF.Copy, scale=-1.0 / H)
                    xc = wp.tile([BLK, BLK], F32, name="xc", tag="xc")
                    nc.vector.tensor_scalar(out=xc[:], in0=r[:], scalar1=negm[:, 0:1],
                                            scalar2=None, op0=ALU.add)
                    sq = wp.tile([BLK, BLK], F32, name="sq", tag="sq")
                    vs = sp.tile([BLK, 1], F32, name="vs", tag="vs")
                    nc.scalar.activation(sq[:], xc[:], AF.Square, accum_out=vs[:])
                    std = sp.tile([BLK, 1], F32, name="std", tag="std")
                    nc.scalar.activation(std[:], vs[:], AF.Sqrt, scale=1.0 / H, bias=1e-5)
                    rstd = sp.tile([BLK, 1], F32, name="rstd", tag="rstd")
                    nc.vector.reciprocal(rstd[:], std[:])
                    nc.vector.tensor_scalar(out=xc[:], in0=xc[:], scalar1=rstd[:, 0:1],
                                            scalar2=None, op0=ALU.mult)
                    nc.vector.tensor_tensor(out=xc[:], in0=xc[:], in1=grep[:, l, :], op=ALU.mult)
                    nc.vector.tensor_tensor(out=h_own[:, w, :], in0=xc[:], in1=brep[:, l, :],
                                            op=ALU.add)
                    hb = wp.tile([BLK, H], BF16, name="hb", tag="hb")
                    nc.vector.tensor_copy(hb[:], h_own[:, w, :])
                    nc.sync.dma_start(own_slice[w * BLK:(w + 1) * BLK, :], hb[:])

                if SKIP != "coll":
                    nc.gpsimd.collective_compute(
                        "AllGather", ALU.bypass, replica_groups=RG,
                        ins=[own_slice[:]], outs=[h_table[:]])

            # ---- P3: output projection (node-major rows per core)
            for w in range(NBLK_CORE):
                tp = ps2.tile([BLK, BLK], F32, name="tp", tag="big", bufs=3)
                nc.tensor.transpose(tp[:], h_own[:, w, :], ident[:])
                hT = wp.tile([BLK, BLK], BF16, name="hT", tag="hT")
                nc.vector.tensor_copy(hT[:], tp[:])
                op_ = ps2.tile([BLK, OUT], F32, name="op_", tag="numT", bufs=2)
                nc.tensor.matmul(op_[:], hT[:], outw[:], start=True, stop=True)
                o_f = wp.tile([BLK, OUT], F32, name="o_f", tag="o_sb")
                nc.vector.tensor_tensor(out=o_f[:], in0=op_[:], in1=outbrep[:], op=ALU.add)
                # per-row symmetric int8 quantization (HW converts round-to-nearest)
                oab = wp.tile([BLK, OUT], F32, name="oab", tag="qf")
                nc.scalar.activation(oab[:], o_f[:], AF.Abs)
                mxt = sp.tile([BLK, 1], F32, name="mxt", tag="mxt")
                nc.vector.tensor_reduce(mxt[:], oab[:], axis=mybir.AxisListType.X,
                                        op=ALU.max)
                nc.vector.tensor_scalar(out=mxt[:], in0=mxt[:], scalar1=1e-20,
                                        scalar2=None, op0=ALU.add)
                rs = sp.tile([BLK, 1], F32, name="rs127", tag="rs127")
                nc.vector.reciprocal(rs[:], mxt[:])
                nc.vector.tensor_scalar(out=rs[:], in0=rs[:], scalar1=127.0,
                                        scalar2=None, op0=ALU.mult)
                qf = wp.tile([BLK, OUT], F32, name="qf", tag="qf")
                nc.vector.tensor_scalar(out=qf[:], in0=o_f[:], scalar1=rs[:, 0:1],
                                        scalar2=None, op0=ALU.mult)
                oq = wp.tile([BLK, OUT], I8, name="oq", tag="oq")
                nc.vector.tensor_copy(oq[:], qf[:])
                sc = sp.tile([BLK, 1], F32, name="sc", tag="sc")
                nc.vector.tensor_scalar(out=sc[:], in0=mxt[:], scalar1=1.0 / 127.0,
                                        scalar2=None, op0=ALU.mult)
                nc.sync.dma_start(out_d[w * BLK:(w + 1) * BLK, 0:OUT], oq[:])
                nc.sync.dma_start(out_d[w * BLK:(w + 1) * BLK, OUT:OUT + 4],
                                  sc[:].bitcast(mybir.dt.int8))

    nc.compile()
    return nc


# -------------------------------------------------------------------- driver
_KCACHE = {}
_LAST_IN_MAPS = None
_JAX_CACHE_SET = False


def _setup_jax_cache():
    global _JAX_CACHE_SET
    if _JAX_CACHE_SET:
        return
    _JAX_CACHE_SET = True
    try:
        import jax
        jax.config.update("jax_compilation_cache_dir", "/tmp/jax_bass_cache")
        jax.config.update("jax_persistent_cache_min_compile_time_secs", 0.0)
        jax.config.update("jax_persistent_cache_min_entry_size_bytes", -1)
    except Exception:
        pass


def kernel(x, edge_index, edge_attr, vnf_context, node_w, node_b, ea_proj_w, ea_proj_b,
           vnf_w, vnf_b, att1_w, att1_b, att2_w, att2_b, att3_w, att3_b,
           gat_lin_w, gat_att_src, gat_att_dst, gat_lin_edge_w, gat_att_edge, gat_bias,
           ln_scale, ln_bias, out_w, out_b):
    _setup_jax_cache()
    from concourse.bass_utils import run_bass_kernel_spmd

    inp = dict(x=x, edge_index=edge_index, edge_attr=edge_attr, vnf_context=vnf_context,
               node_w=node_w, node_b=node_b, ea_proj_w=ea_proj_w, ea_proj_b=ea_proj_b,
               vnf_w=vnf_w, vnf_b=vnf_b, att1_w=att1_w, att1_b=att1_b, att2_w=att2_w,
               att2_b=att2_b, att3_w=att3_w, att3_b=att3_b, gat_lin_w=gat_lin_w,
               gat_att_src=gat_att_src, gat_att_dst=gat_att_dst,
               gat_lin_edge_w=gat_lin_edge_w, gat_att_edge=gat_att_edge,
               gat_bias=gat_bias, ln_scale=ln_scale, ln_bias=ln_bias,
               out_w=out_w, out_b=out_b)

    cores, meta = _build_graph(edge_index)
    gw = _derive_weights(inp, meta, cores)

    import ml_dtypes
    wfull = meta["wfull"]
    blob_f32 = np.ascontiguousarray(np.concatenate(
        [gw["rows"].ravel(), gw["b2a3"].astype(np.float32).ravel()]
    ), dtype=np.float32)
    in_maps = []
    for c in range(NCORES):
        blob_bf = np.ascontiguousarray(np.concatenate([
            cores[c]["dstloc"].ravel(),
            cores[c]["xT_own"].ravel(),
            wfull[16 * c:16 * (c + 1), :].ravel(),
            gw["ea_proj_w"].ravel()]).astype(ml_dtypes.bfloat16))
        ball = np.concatenate([
            blob_f32.view(np.int8),
            blob_bf.view(np.int8),
            np.ascontiguousarray(cores[c]["sd_tab"]).view(np.int8).ravel(),
            np.ascontiguousarray(cores[c]["eaT"]).view(np.int8).ravel(),
        ])[None, :]
        in_maps.append(dict(ball=np.ascontiguousarray(ball)))

    key = (meta["T"], meta["cht_pad"], meta["b3"])
    if key not in _KCACHE:
        nc = _build_kernel(meta)
        bir = nc.to_json_bytes()
        nc.to_json_bytes = lambda: bir
        _KCACHE[key] = nc
    nc = _KCACHE[key]

    global _LAST_IN_MAPS
    _LAST_IN_MAPS = in_maps
    res = run_bass_kernel_spmd(nc, in_maps, list(range(NCORES)))
    bounds = meta["bounds"]
    out = np.zeros((N, OUT), dtype=np.float32)
    for c in range(NCORES):
        nb = bounds[c + 1] - bounds[c]
        r0, r1 = bounds[c] * BLK, min(bounds[c + 1] * BLK, N)
        blob = res.results[c]["out"][:r1 - r0]
        q = blob[:, :OUT].astype(np.float32)
        s = np.ascontiguousarray(blob[:, OUT:OUT + 4]).view(np.float32)
        out[r0:r1] = q * s
    return out
